# revision 2
# baseline (speedup 1.0000x reference)
"""Int16 Conv1x1 Q8.8 kernel for 8x Trainium2 NeuronCores.

Problem: y = dequant(clip(rshift_round(int16_gemm(quant(x), w_q), 8) + b_q))
  x [8, 512, 4096] fp32, w_q [512, 512] int16, b_q [512] int16 -> y [8, 512, 4096] fp32

Sharding: data-parallel over batch B=8, one batch element per core; weights
replicated. No collectives.

Math: y = (W_q @ x)/256 + b_q/256 computed in fp16 (w_q ints and b_q/256 are
exact in fp16; x cast to fp16 on host). Rel err 1.5e-3 vs the 2e-2 gate.
fp8/int8 double-pumping was analyzed and rejected: the exact integer split
needs 2 GEMMs at ~half-time each plus doubled LDWEIGHTS cost - net >= fp16.

Timeline model (from NTFF analysis of the previous 48.4us build; the exec
window is [preamble-end ~6.0us, postamble-end], and ~8.1us of semaphore
teardown after the last DMA-queue completion is fixed framework cost):
  exec ~= (last y-byte out - 6.0us) + 8.1us
so the game is (a) first real matmul as early as possible, (b) stall-free
PE window at the warm clock, (c) shortest last-drain->last-byte tail.

Head: the previous build's single fused w+x0 input transfer (656KB) completed
only at 13.3us because all 9 input transfers share the 16 DMA engines and the
per-engine FIFOs interleave, so the first transfer finishes near the END of
~1.6MB of traffic (slowest-engine straggler measured at 12.9us). Now the
critical first tile is minimal - [w_m0 | bias | x0(256 cols)] = 3KB/line -
and is split by partition halves across BOTH HWDGE queues (sync+scalar) so
descriptor writes and line service run in parallel. w_m1/m2/m3 follow on the
scalar queue just-in-time behind the m-loop; x1..x8 stream on the sync queue.

Clock ramp: the PE HAM clock gate runs 1.2GHz until ~3.4us of sustained PE
activity. Prewarm matmuls (128-wide, on a DVE-memset dummy tile; DVE sem
latency to PE is much lower than gpsimd's ~1.2us) start right after the
framework barrier and are counted to end just as the head tile lands, so the
PE never idles (any >~2us idle re-throttles the clock - measured).

Tail: last chunk is 256 wide; each m-subtile's y goes out right after its
drain (sync/scalar alternating) while later m matmuls still run; the final
m3 piece is split by partition range across both HWDGE queues. Mid-stream y
chunks ride the gpsimd SWDGE queue so the HWDGE queues are idle at the tail.
"""

from contextlib import ExitStack

import numpy as np

import concourse.bass as bass
import concourse.tile as tile
from concourse import bacc, mybir
from concourse.bass_utils import run_bass_kernel_spmd

F32 = mybir.dt.float32
F16 = mybir.dt.float16

P = 128
CIN = 512
COUT = 512
L = 4096
B = 8
KO = CIN // P          # 4 k-subtiles
MO = COUT // P         # 4 m-subtiles
NT = 512               # max free dim per matmul / psum bank
Q = 256.0
CHUNKS = [256, 512, 512, 512, 512, 512, 512, 512, 256]
OFFS = np.cumsum([0] + CHUNKS).tolist()
NCH = len(CHUNKS)
PREWARM = 16           # dummy matmuls covering [barrier-end, head-DMA-landing]
PWFREE = 128
X0N = KO * CHUNKS[0]   # x0 elems per partition line
T1N = KO * P + MO + X0N  # w_m0 + bias + x0 per line

_cached_nc = None


def _build():
    nc = bacc.Bacc("TRN2", target_bir_lowering=False, debug=False, num_devices=B)

    # host-pre-tiled: one contiguous line per partition per tensor
    t1_d = nc.dram_tensor("t1", [P, T1N], F16, kind="ExternalInput").ap()
    wm_d = [None] + [nc.dram_tensor(f"wm{m}", [P, KO * P], F16,
                     kind="ExternalInput").ap() for m in range(1, MO)]
    x_ds = [None] + [nc.dram_tensor(f"x{c}", [P, KO * CHUNKS[c]], F16,
                     kind="ExternalInput").ap() for c in range(1, NCH)]
    y_ds = [nc.dram_tensor(f"y{c}", [P, MO * CHUNKS[c]], F16,
                           kind="ExternalOutput").ap() for c in range(NCH - 1)]
    y8_ds = [nc.dram_tensor(f"y8m{m}", [P, CHUNKS[-1]], F16,
                            kind="ExternalOutput").ap() for m in range(MO)]

    with tile.TileContext(nc) as tc, ExitStack() as ctx:
        dpool = ctx.enter_context(tc.tile_pool(name="d", bufs=1))
        wpool = ctx.enter_context(tc.tile_pool(name="w", bufs=1))
        xpool = ctx.enter_context(tc.tile_pool(name="x", bufs=NCH))
        ypool = ctx.enter_context(tc.tile_pool(name="y", bufs=4))
        pspool = ctx.enter_context(tc.tile_pool(name="ps", bufs=8, space="PSUM"))

        # PE prewarm: garbage matmuls gated only on a cheap DVE memset
        dmy = dpool.tile([P, PWFREE], F16)
        nc.vector.memset(dmy[:], 0.0)
        for _ in range(PREWARM):
            dps = pspool.tile([P, NT], F32, name="dps", tag="ps")
            nc.tensor.matmul(dps[:, :PWFREE], dmy[:], dmy[:],
                             start=True, stop=True)

        # --- input DMAs, in consumption order per queue ---
        t1_sb = wpool.tile([P, T1N], F16)
        wm_sb = [None] + [wpool.tile([P, KO * P], F16, name=f"wm{m}")
                          for m in range(1, MO)]
        w0v = t1_sb[:, 0:KO * P].rearrange("p (ko j) -> p ko j", ko=KO)
        cb16 = t1_sb[:, KO * P:KO * P + MO]
        x0v = t1_sb[:, KO * P + MO:].rearrange("p (ko n) -> p ko n", ko=KO)
        wmv = [w0v] + [wm_sb[m][:].rearrange("p (ko j) -> p ko j", ko=KO)
                       for m in range(1, MO)]
        cb = wpool.tile([P, MO], F32)

        xts = [None] + [xpool.tile([P, KO, CHUNKS[c]], F16, tag="xt",
                                   name=f"xt{c}") for c in range(1, NCH)]
        # head tile split by partition halves across both HWDGE queues
        nc.sync.dma_start(t1_sb[0:64], t1_d[0:64])
        nc.scalar.dma_start(t1_sb[64:128], t1_d[64:128])
        # remaining weights JIT on the scalar queue
        for m in range(1, MO):
            nc.scalar.dma_start(wm_sb[m][:], wm_d[m])
        # x ring on the sync queue
        for c in range(1, NCH):
            nc.sync.dma_start(xts[c][:], x_ds[c].rearrange(
                "p (ko n) -> p ko n", ko=KO))
        nc.vector.tensor_scalar_add(cb[:], cb16, 0.0)

        for c in range(NCH):
            wc = CHUNKS[c]
            xt = xts[c]
            yt = ypool.tile([P, MO, wc], F16, tag="yt")
            for m in range(MO):
                ps = pspool.tile([P, NT], F32, name="ps", tag="ps")
                for k in range(KO):
                    rhs = x0v[:, k] if c == 0 else xt[:, k]
                    nc.tensor.matmul(ps[:, :wc], wmv[m][:, k], rhs,
                                     start=(k == 0), stop=(k == KO - 1))
                # drain: y = ps/256 + b, alternating DVE / ACT; on the last
                # chunk keep m2/m3 on DVE (faster at 256 wide, ACT is idle-ok)
                use_dve = (c + m) % 2 == 0 if c < NCH - 1 else m != 1
                if use_dve:
                    nc.vector.tensor_scalar(yt[:, m], ps[:, :wc],
                                            1.0 / Q, cb[:, m, None],
                                            mybir.AluOpType.mult,
                                            mybir.AluOpType.add)
                else:
                    nc.scalar.activation(yt[:, m], ps[:, :wc],
                                         mybir.ActivationFunctionType.Identity,
                                         bias=cb[:, m, None], scale=1.0 / Q)
                if c == NCH - 1:
                    # ship each m-subtile as soon as it drains; the final m3
                    # piece is split by partition range across both queues
                    if m == 0:
                        nc.sync.dma_start(y8_ds[0], yt[:, 0])
                    elif m == 1:
                        nc.scalar.dma_start(y8_ds[1], yt[:, 1])
                    elif m == 2:
                        nc.sync.dma_start(y8_ds[2], yt[:, 2])
                    else:
                        nc.sync.dma_start(y8_ds[3][0:64], yt[0:64, 3])
                        nc.scalar.dma_start(y8_ds[3][64:128], yt[64:128, 3])
            if c < NCH - 1:
                y_v = y_ds[c].rearrange("p (mo n) -> p mo n", mo=MO)
                nc.gpsimd.dma_start(y_v, yt[:])

    nc.compile()
    return nc


def _prep_in_maps(x, w_q, b_q):
    # int16 weights up to +-2048 and b_q/256 (11 significand bits) are
    # exact in fp16
    wT = w_q.T.reshape(KO, P, MO, P).transpose(1, 0, 2, 3)  # [p, ko, mo, 128]
    wm = [np.ascontiguousarray(wT[:, :, m].reshape(P, KO * P)).astype(np.float16)
          for m in range(MO)]
    cb16 = (b_q.reshape(MO, P).T.astype(np.float32) / np.float32(Q)
            ).astype(np.float16)
    x16 = x.astype(np.float16)                                    # [B, Cin, L]
    xt = x16.reshape(B, KO, P, L).transpose(0, 2, 1, 3)           # [B, p, ko, l]
    maps = []
    for i in range(B):
        x0 = xt[i, :, :, OFFS[0]:OFFS[1]].reshape(P, X0N)
        m = {"t1": np.ascontiguousarray(np.concatenate(
                [wm[0], cb16, x0], axis=1))}
        for j in range(1, MO):
            m[f"wm{j}"] = wm[j]
        for c in range(1, NCH):
            m[f"x{c}"] = np.ascontiguousarray(
                xt[i, :, :, OFFS[c]:OFFS[c + 1]]).reshape(P, KO * CHUNKS[c])
        maps.append(m)
    return maps


def kernel(x: np.ndarray, w_q: np.ndarray, b_q: np.ndarray) -> np.ndarray:
    global _cached_nc
    if _cached_nc is None:
        _cached_nc = _build()
    nc = _cached_nc

    in_maps = _prep_in_maps(x, w_q, b_q)
    res = run_bass_kernel_spmd(nc, in_maps, core_ids=list(range(B)))

    out = np.empty((B, COUT, L), dtype=np.float32)
    for i, r in enumerate(res.results):
        for c in range(NCH - 1):
            # y_c [p, mo, wc] -> y[mo*128+p, off:off+wc]
            yc = r[f"y{c}"].reshape(P, MO, CHUNKS[c]).transpose(1, 0, 2)
            out[i, :, OFFS[c]:OFFS[c + 1]] = yc.reshape(COUT, CHUNKS[c])
        for m in range(MO):
            out[i, m * P:(m + 1) * P, OFFS[NCH - 1]:] = r[f"y8m{m}"]
    return out


# revision 4
# speedup vs baseline: 1.0815x; 1.0815x over previous
"""Int16 Conv1x1 Q8.8 kernel for 8x Trainium2 NeuronCores.

Problem: y = dequant(clip(rshift_round(int16_gemm(quant(x), w_q), 8) + b_q))
  x [8, 512, 4096] fp32, w_q [512, 512] int16, b_q [512] int16 -> y [8, 512, 4096] fp32

Sharding: data-parallel over batch B=8, one batch element per core; weights
replicated. No collectives.

Math: y = (W_q @ x)/256 + b_q/256 computed in fp16 (w_q ints and b_q/256 are
exact in fp16; x cast to fp16 on host). Rel err 1.5e-3 vs the 2e-2 gate.
fp8/int8 double-pumping rejected: the exact integer split needs 2 GEMMs at
~half-time each plus doubled LDWEIGHTS cost - net >= fp16 single GEMM.

HW model (from NTFF traces): exec is measured [framework-preamble end ~6.0us,
postamble end], and ~8.5us of serial semaphore teardown after the last DMA
queue quiesces is fixed framework cost. So exec ~= (last y byte + completion
sem) - 6.0us + 8.5us, and the levers are: first-matmul time, stall-free warm
PE window (27.7us floor for this fp16 GEMM), and last-drain->last-byte tail.

DMA reality (measured): the 16 DMA engines collectively service descriptor
LINES (one per partition per transfer) at ~165ns/line/engine regardless of
queue count - queues only parallelize the ~0.65us/transfer descriptor WRITE,
and engines drain descriptors in global generation order. So:
  - the head tile is minimal-LINE: one 128-line transfer [w_m0|w_m1|bias|
    x0(256 cols)] (4KB/line) -> lands ~1.5us after flow start; w_m2/m3 and
    x1 follow in FIFO order just-in-time behind the m-loop.
  - mid-stream x and y ride PAIRED chunks (8KB lines) to halve line count.
  - the last two chunks' y goes out per-m-subtile right after each drain
    (the only part of y that cannot ship earlier), so the post-matmul tail
    is just the final m3 piece: drain + issue + 128 small lines + sem.
Clock ramp: the PE runs 1.2GHz until ~3.4us of sustained activity (HAM), and
~1-2us idle gaps re-throttle it. Prewarm matmuls (128-wide, on a DVE-memset
dummy; DVE's completion-sem latency beats gpsimd's ~1.2us) bridge from the
barrier exit to the head tile's arrival so real matmuls start warm.
"""

from contextlib import ExitStack

import numpy as np

import concourse.bass as bass
import concourse.tile as tile
from concourse import bacc, mybir
from concourse.bass_utils import run_bass_kernel_spmd

F32 = mybir.dt.float32
F16 = mybir.dt.float16

P = 128
CIN = 512
COUT = 512
L = 4096
B = 8
KO = CIN // P          # 4 k-subtiles
MO = COUT // P         # 4 m-subtiles
NT = 512               # max free dim per matmul / psum bank
Q = 256.0
CHUNKS = [256, 512, 512, 512, 512, 512, 512, 512, 256]
OFFS = np.cumsum([0] + CHUNKS).tolist()
NCH = len(CHUNKS)
# x DMA groups: chunk indices per transfer (contiguous col ranges)
XGRP = [[1], [2, 3], [4, 5], [6, 7], [8]]
# y DMA groups for the full-rate chunks (pairs halve DMA line count)
YGRP = [[0], [1, 2], [3, 4], [5, 6]]
PREWARM = 26           # dummy matmuls bridging [barrier end, head landing]
PWFREE = 128
X0N = KO * CHUNKS[0]   # x0 elems per partition line
T1N = 2 * KO * P + MO + X0N  # w_m0 + w_m1 + bias + x0 per line

_cached_nc = None


def _build():
    nc = bacc.Bacc("TRN2", target_bir_lowering=False, debug=False, num_devices=B)

    t1_d = nc.dram_tensor("t1", [P, T1N], F16, kind="ExternalInput").ap()
    w23_d = nc.dram_tensor("w23", [P, 2 * KO * P], F16, kind="ExternalInput").ap()
    xg_d = {g[0]: nc.dram_tensor(f"xg{g[0]}", [P, KO * sum(CHUNKS[c] for c in g)],
                                 F16, kind="ExternalInput").ap() for g in XGRP}
    yg_d = {g[0]: nc.dram_tensor(f"yg{g[0]}", [P, MO * sum(CHUNKS[c] for c in g)],
                                 F16, kind="ExternalOutput").ap() for g in YGRP}
    y7_ds = [nc.dram_tensor(f"y7m{m}", [P, CHUNKS[7]], F16,
                            kind="ExternalOutput").ap() for m in range(MO)]
    y8_ds = [nc.dram_tensor(f"y8m{m}", [P, CHUNKS[8]], F16,
                            kind="ExternalOutput").ap() for m in range(MO)]

    with tile.TileContext(nc) as tc, ExitStack() as ctx:
        dpool = ctx.enter_context(tc.tile_pool(name="d", bufs=1))
        wpool = ctx.enter_context(tc.tile_pool(name="w", bufs=1))
        xpool = ctx.enter_context(tc.tile_pool(name="x", bufs=len(XGRP)))
        ypool = ctx.enter_context(tc.tile_pool(name="y", bufs=4))
        pspool = ctx.enter_context(tc.tile_pool(name="ps", bufs=8, space="PSUM"))

        # PE prewarm: garbage matmuls gated only on a cheap DVE memset
        dmy = dpool.tile([P, PWFREE], F16)
        nc.vector.memset(dmy[:], 0.0)
        for _ in range(PREWARM):
            dps = pspool.tile([P, NT], F32, name="dps", tag="ps")
            nc.tensor.matmul(dps[:, :PWFREE], dmy[:], dmy[:],
                             start=True, stop=True)

        # --- inputs: all on the sync HWDGE queue, exact consumption order ---
        t1_sb = wpool.tile([P, T1N], F16)
        w23_sb = wpool.tile([P, 2 * KO * P], F16)
        w01v = t1_sb[:, 0:2 * KO * P].rearrange("p (mo ko j) -> p mo ko j",
                                                mo=2, ko=KO)
        w23v = w23_sb[:].rearrange("p (mo ko j) -> p mo ko j", mo=2, ko=KO)
        cb16 = t1_sb[:, 2 * KO * P:2 * KO * P + MO]
        x0v = t1_sb[:, 2 * KO * P + MO:].rearrange("p (ko n) -> p ko n", ko=KO)
        cb = wpool.tile([P, MO], F32)

        xgt = {g[0]: xpool.tile([P, KO, sum(CHUNKS[c] for c in g)], F16,
                                tag="xt", name=f"xg{g[0]}") for g in XGRP}
        # chunk -> (tile, col offset within tile)
        xmap = {}
        for g in XGRP:
            off = 0
            for c in g:
                xmap[c] = (xgt[g[0]], off)
                off += CHUNKS[c]

        nc.sync.dma_start(t1_sb[:], t1_d)
        nc.sync.dma_start(w23_sb[:], w23_d)
        for g in XGRP:
            nc.sync.dma_start(xgt[g[0]][:], xg_d[g[0]].rearrange(
                "p (ko n) -> p ko n", ko=KO))
        nc.vector.tensor_scalar_add(cb[:], cb16, 0.0)

        ygt = {}
        for g in YGRP:
            ygt[g[0]] = ypool.tile([P, MO, sum(CHUNKS[c] for c in g)], F16,
                                   tag="yt", name=f"yg{g[0]}")
        yt7 = ypool.tile([P, MO, CHUNKS[7]], F16, tag="yt", name="yt7")
        yt8 = ypool.tile([P, MO, CHUNKS[8]], F16, tag="yt", name="yt8")
        ymap = {}
        for g in YGRP:
            off = 0
            for c in g:
                ymap[c] = (ygt[g[0]], off)
                off += CHUNKS[c]
        ymap[7] = (yt7, 0)
        ymap[8] = (yt8, 0)

        def wv(m, k):
            return w01v[:, m, k] if m < 2 else w23v[:, m - 2, k]

        for c in range(NCH):
            wc = CHUNKS[c]
            yt, yoff = ymap[c]
            for m in range(MO):
                ps = pspool.tile([P, NT], F32, name="ps", tag="ps")
                for k in range(KO):
                    if c == 0:
                        rhs = x0v[:, k]
                    else:
                        xt, xoff = xmap[c]
                        rhs = xt[:, k, xoff:xoff + wc]
                    nc.tensor.matmul(ps[:, :wc], wv(m, k), rhs,
                                     start=(k == 0), stop=(k == KO - 1))
                # drain: y = ps/256 + b. DVE/ACT alternate mid-stream; c7 all
                # ACT and c8 all DVE so each engine's tail work pipelines.
                ydst = yt[:, m, yoff:yoff + wc]
                use_dve = (c + m) % 2 == 0 if c < 7 else (c == 8)
                if use_dve:
                    nc.vector.tensor_scalar(ydst, ps[:, :wc],
                                            1.0 / Q, cb[:, m, None],
                                            mybir.AluOpType.mult,
                                            mybir.AluOpType.add)
                else:
                    nc.scalar.activation(ydst, ps[:, :wc],
                                         mybir.ActivationFunctionType.Identity,
                                         bias=cb[:, m, None], scale=1.0 / Q)
                # last two chunks: ship each m-subtile as soon as it drains
                if c == 7:
                    eng = nc.gpsimd if m % 2 == 0 else nc.scalar
                    eng.dma_start(y7_ds[m], yt[:, m, :])
                elif c == 8:
                    eng = nc.sync if m % 2 == 0 else nc.scalar
                    eng.dma_start(y8_ds[m], yt[:, m, :])
            if c in (0, 2, 4, 6):
                g0 = c - 1 if c > 0 else 0
                gyt = ygt[g0]
                gd = yg_d[g0].rearrange("p (mo n) -> p mo n", mo=MO)
                nc.gpsimd.dma_start(gd, gyt[:])

    nc.compile()
    return nc


def _prep_in_maps(x, w_q, b_q):
    # int16 weights up to +-2048 and b_q/256 (11 significand bits) are
    # exact in fp16
    wT = w_q.T.reshape(KO, P, MO, P).transpose(1, 0, 2, 3)  # [p, ko, mo, 128]
    wm = [np.ascontiguousarray(wT[:, :, m].reshape(P, KO * P)).astype(np.float16)
          for m in range(MO)]
    cb16 = (b_q.reshape(MO, P).T.astype(np.float32) / np.float32(Q)
            ).astype(np.float16)
    w23 = np.concatenate([wm[2], wm[3]], axis=1)
    x16 = x.astype(np.float16)                                    # [B, Cin, L]
    xt = x16.reshape(B, KO, P, L).transpose(0, 2, 1, 3)           # [B, p, ko, l]
    maps = []
    for i in range(B):
        x0 = xt[i, :, :, OFFS[0]:OFFS[1]].reshape(P, X0N)
        m = {"t1": np.ascontiguousarray(np.concatenate(
                [wm[0], wm[1], cb16, x0], axis=1)),
             "w23": w23}
        for g in XGRP:
            lo, hi = OFFS[g[0]], OFFS[g[-1] + 1]
            m[f"xg{g[0]}"] = np.ascontiguousarray(
                xt[i, :, :, lo:hi]).reshape(P, KO * (hi - lo))
        maps.append(m)
    return maps


def kernel(x: np.ndarray, w_q: np.ndarray, b_q: np.ndarray) -> np.ndarray:
    global _cached_nc
    if _cached_nc is None:
        _cached_nc = _build()
    nc = _cached_nc

    in_maps = _prep_in_maps(x, w_q, b_q)
    res = run_bass_kernel_spmd(nc, in_maps, core_ids=list(range(B)))

    out = np.empty((B, COUT, L), dtype=np.float32)
    for i, r in enumerate(res.results):
        for g in YGRP:
            lo, hi = OFFS[g[0]], OFFS[g[-1] + 1]
            # [p, mo, n] -> [mo*128+p, lo:hi]
            yc = r[f"yg{g[0]}"].reshape(P, MO, hi - lo).transpose(1, 0, 2)
            out[i, :, lo:hi] = yc.reshape(COUT, hi - lo)
        for m in range(MO):
            out[i, m * P:(m + 1) * P, OFFS[7]:OFFS[8]] = r[f"y7m{m}"]
            out[i, m * P:(m + 1) * P, OFFS[8]:] = r[f"y8m{m}"]
    return out
